# revision 27
# baseline (speedup 1.0000x reference)
"""LlamaAttention (B=2, S=2048, D=2048, H=16) on 8 Trainium2 NeuronCores.

Sharding: batch x head-group. Core c handles batch b = c // 4 and head group
g = c % 4 (4 heads of 128 dims each -> a 512-wide slice of q/k/v space).
Each core computes q/k/v projections for its slice, attention for its 4
heads, and a partial out-projection (contracting only its 512 dv dims).
Host sums the 4 partials per batch and adds the output bias.

Design notes (engine rates measured from hardware traces):
  - PE is the roofline (1536 matmuls x 512 moving cols ~ 330 us/core); the
    schedule keeps the PE queue dense so it stays at full p-state (trn2 PE
    halves its clock after idle gaps and needs ~3us to ramp back).
  - Everything is bf16 on the device (same PE rate as float32r, half the
    DMA/SBUF, 2x DVE): x and the four weights are converted host-side
    (host time is not on the HW critical path).  Measured rel err ~5e-3
    against the fp32 reference, mostly from bf16 q/k logit noise.
  - x (8MB in bf16) is loaded ONCE and stays resident; q/k and v
    projections both read it from SBUF.  Total HBM traffic ~32MB.
  - x DMAs land chunk-major ([128,512] pieces of the resident [128,2048]
    tiles) so the first projection group starts after ~2MB, not 8MB.
  - q/k scale+bias folded into the PSUM eviction (activation bias; scale
    pre-folded into Wq host-side); v bias via a ones-row matmul into PSUM.
  - scores are computed transposed (keys on partitions) into 2-bank PSUM
    tiles, so exp is a single 1024-wide activation with the additive
    attention mask as its per-partition bias (exact for the general
    [B, S] mask since keys sit on partitions).
  - softmax denominator: bf16 racc accumulated on DVE (2x 16-bit mode),
    one gpsimd partition_all_reduce (sum+broadcast across partitions in
    one op), reciprocal on the SCALAR engine (DVE reciprocal is a 6.5us
    multi-pass op; scalar does it in ~1us), normalization fused into the
    PV-psum eviction on DVE.
  - attention is emitted per (query-block 1024, head); PV matmuls are
    software-pipelined 2 sk-steps behind the score matmuls; the
    out-projection shares the scores' 2-bank PSUM rotation so PSUM is
    exactly 8 banks.
  - the attention inner loop is scalar-exp-paced (~1.25us per 16K-score
    exp), so out-projection eo-groups of query-block qb-1 are interleaved
    into qb's sk loops to fill the PE's deficit, and head 0 of block 0
    gets its scores+exps precomputed during the (pure-PE) v projection.
  - y is evacuated as bf16 (host sums the 4 partials in fp32): full-rate
    fp32 y DMAs contend with the PE's SBUF operand fetches and slowed
    out-projection matmuls to ~427ns.
"""

import os
import numpy as np
import ml_dtypes

import concourse.bass as bass
import concourse.tile as tile
from concourse import bacc, mybir, bass_isa
from concourse import bass_utils

B, S, D = 2, 2048, 2048
NH, HD = 16, 128
N_CORES = 8
HPC = 4                      # heads per core
E = HPC * HD                 # 512: per-core q/k/v width
SCALE = float(HD) ** -0.5
F32 = mybir.dt.float32
BF16 = mybir.dt.bfloat16
MM_DT = BF16                 # matmul input dtype everywhere

P = 128                      # partition tile
ST = S // P                  # 16 s partition-tiles
DTI = D // P                 # 16 d partition-tiles
SB = 512                     # matmul moving-dim block
NCH = S // SB                # 4 s chunks for the projection passes
QW = 1024                    # attention query-block width (2-bank psum)
NQB = S // QW                # 2 query blocks
PIPE = 2                     # PV pipeline lag (sk steps)
MASK_MIN = float(np.finfo(np.float32).min)

BF16_NP = ml_dtypes.bfloat16


def _build():
    nc = bacc.Bacc("TRN2", target_bir_lowering=False, debug=False,
                   num_devices=N_CORES)

    xT = nc.dram_tensor("xT", [D, S], BF16, kind="ExternalInput").ap()
    wqT = nc.dram_tensor("wqT", [D, E], BF16, kind="ExternalInput").ap()
    wkT = nc.dram_tensor("wkT", [D, E], BF16, kind="ExternalInput").ap()
    wvT = nc.dram_tensor("wvT", [D, E], BF16, kind="ExternalInput").ap()
    woT = nc.dram_tensor("woT", [E, D], BF16, kind="ExternalInput").ap()
    maskT = nc.dram_tensor("maskT", [S], F32, kind="ExternalInput").ap()
    bqd = nc.dram_tensor("bq", [E], F32, kind="ExternalInput").ap()
    bkd = nc.dram_tensor("bk", [E], F32, kind="ExternalInput").ap()
    bvd = nc.dram_tensor("bv", [E], BF16, kind="ExternalInput").ap()
    ones1 = nc.dram_tensor("ones1", [P], BF16, kind="ExternalInput").ap()
    yT = nc.dram_tensor("yT", [D, S], BF16, kind="ExternalOutput").ap()

    ACT = mybir.ActivationFunctionType

    with tile.TileContext(nc) as tc:
        with tc.tile_pool(name="persist", bufs=1) as persist, \
             tc.tile_pool(name="xpool", bufs=1) as xpool:
            qT = [persist.tile([P, S], BF16, name=f"qT{h}", tag=f"qT{h}")
                  for h in range(HPC)]
            kT = [persist.tile([P, S], BF16, name=f"kT{h}", tag=f"kT{h}")
                  for h in range(HPC)]
            mask_sb = persist.tile([P, ST], F32, name="mask_sb", tag="mask")
            bq_sb = persist.tile([P, HPC], F32, name="bq_sb", tag="bq")
            bk_sb = persist.tile([P, HPC], F32, name="bk_sb", tag="bk")
            bv_row = persist.tile([1, E], BF16, name="bv_row", tag="bv")
            ones_rp = persist.tile([1, P], BF16, name="ones_rp", tag="onesr")
            nc.sync.dma_start(mask_sb[:, :],
                              maskT.rearrange("(t p) -> p t", p=P))
            nc.sync.dma_start(bq_sb[:, :],
                              bqd.rearrange("(t p) -> p t", p=P))
            nc.sync.dma_start(bk_sb[:, :],
                              bkd.rearrange("(t p) -> p t", p=P))
            nc.sync.dma_start(bv_row[:, :],
                              bvd.rearrange("(a e) -> a e", a=1))
            nc.sync.dma_start(ones_rp[:, :],
                              ones1.rearrange("(a e) -> a e", a=1))

            # Resident x: 16 tiles [128, 2048] bf16 (64KB/partition).
            xsb = [xpool.tile([P, S], BF16, name=f"x_{dt}", tag=f"x_{dt}")
                   for dt in range(DTI)]

            def load_x_chunk(ch):
                c0 = ch * SB
                for dt in range(DTI):
                    nc.sync.dma_start(xsb[dt][:, c0:c0 + SB],
                                      xT[dt * P:(dt + 1) * P, c0:c0 + SB])

            # ---------------- Phase A: q + k projections --------------------
            # qT[e, s] = wq.T-slice @ x (+bq, scale pre-folded); kT likewise.
            with nc.named_scope("proj_qk"), \
                 tc.tile_pool(name="wqk", bufs=1) as wpool, \
                 tc.tile_pool(name="ps_a", bufs=6, space="PSUM") as psa:
                wq_sb, wk_sb = [], []
                # interleave wq/x-chunk0 DMAs so chunk-0 matmuls start early
                for dt in range(DTI):
                    wq_t = wpool.tile([P, E], BF16, name=f"wq_{dt}",
                                      tag=f"wq_{dt}")
                    nc.sync.dma_start(wq_t[:, 0:P],
                                      wqT[dt * P:(dt + 1) * P, 0:P])
                    wq_sb.append(wq_t)
                    nc.sync.dma_start(xsb[dt][:, 0:SB],
                                      xT[dt * P:(dt + 1) * P, 0:SB])
                for dt in range(DTI):
                    nc.sync.dma_start(wq_sb[dt][:, P:E],
                                      wqT[dt * P:(dt + 1) * P, P:E])
                for dt in range(DTI):
                    wk_t = wpool.tile([P, E], BF16, name=f"wk_{dt}",
                                      tag=f"wk_{dt}")
                    nc.sync.dma_start(wk_t[:, :],
                                      wkT[dt * P:(dt + 1) * P, :])
                    wk_sb.append(wk_t)
                load_x_chunk(1)
                for ch in range(NCH):
                    c0 = ch * SB
                    if 2 <= ch + 2 < NCH:
                        load_x_chunk(ch + 2)
                    for wsb, outT, bsb in ((wq_sb, qT, bq_sb),
                                           (wk_sb, kT, bk_sb)):
                        for et in range(HPC):
                            ps = psa.tile([P, SB], F32, name="ps_at")
                            if ch == 0 and et == 0 and wsb is wq_sb:
                                # first group split into 256-wide halves so
                                # the PE starts before the full chunk lands
                                for hb in range(2):
                                    b0 = hb * HB
                                    for dt in range(DTI):
                                        nc.tensor.matmul(
                                            ps[:, b0:b0 + HB],
                                            wsb[dt][:, 0:P],
                                            xsb[dt][:, b0:b0 + HB],
                                            start=(dt == 0),
                                            stop=(dt == DTI - 1))
                            else:
                                for dt in range(DTI):
                                    nc.tensor.matmul(
                                        ps[:, :],
                                        wsb[dt][:, et * P:(et + 1) * P],
                                        xsb[dt][:, c0:c0 + SB],
                                        start=(dt == 0), stop=(dt == DTI - 1))
                            nc.scalar.activation(
                                outT[et][:, c0:c0 + SB], ps[:, :],
                                ACT.Identity, bias=bsb[:, et:et + 1])

            with tc.tile_pool(name="late", bufs=1) as late:
                vv = [late.tile([P, E], BF16, name=f"v{st}", tag=f"v{st}")
                      for st in range(ST)]
                wo_sb = [late.tile([P, D], BF16, name=f"wo_{dv}",
                                   tag=f"wo_{dv}") for dv in range(HPC)]

                # expp/raccp/ps2 open before phase B: the v projection is
                # pure PE work, so head 0 of query-block 0 gets its scores
                # and exps computed there (the scalar is otherwise idle),
                # collapsing its phase-C block to just the PV matmuls.
                exs0 = [None] * ST
                with tc.tile_pool(name="expp", bufs=18) as expp, \
                     tc.tile_pool(name="raccp", bufs=2) as raccp, \
                     tc.tile_pool(name="ps2", bufs=2, space="PSUM") as ps2:
                  racc0 = raccp.tile([P, QW], BF16, name="racc", tag="racc")
                  # ---------------- Phase B: v projection (natural) ---------
                  # v[s, e] = x-slice.T @ wv + bv; x already resident.
                  with nc.named_scope("proj_v"), \
                       tc.tile_pool(name="wv", bufs=1) as wvpool, \
                       tc.tile_pool(name="ps_v", bufs=4, space="PSUM") as psv:
                    wv_sb = []
                    for dt in range(DTI):
                        wv_t = wvpool.tile([P, E], BF16, name=f"wv_{dt}",
                                           tag=f"wv_{dt}")
                        nc.sync.dma_start(wv_t[:, :],
                                          wvT[dt * P:(dt + 1) * P, :])
                        wv_sb.append(wv_t)
                    for dv in range(HPC):
                        nc.sync.dma_start(wo_sb[dv][:, :],
                                          woT[dv * P:(dv + 1) * P, :])
                    for st in range(ST):
                        s0 = st * P
                        ps = psv.tile([P, E], F32, name="ps_vt")
                        for dt in range(DTI):
                            nc.tensor.matmul(
                                ps[:, :],
                                xsb[dt][:, s0:s0 + P],
                                wv_sb[dt][:, :],
                                start=(dt == 0), stop=False)
                        nc.tensor.matmul(
                            ps[:, :], ones_rp[0:1, :], bv_row[0:1, :],
                            start=False, stop=True)
                        nc.vector.tensor_copy(vv[st][:, :], ps[:, :])
                        # (qb0, h0) scores + exp ride along: scalar work
                        # under phase B's PE-bound v projection
                        ps_s = ps2.tile([P, QW], F32, name="ps_sc",
                                        tag="ps2")
                        nc.tensor.matmul(
                            ps_s[:, 0:SB],
                            kT[0][:, s0:s0 + P], qT[0][:, 0:SB],
                            start=True, stop=True)
                        nc.tensor.matmul(
                            ps_s[:, SB:QW],
                            kT[0][:, s0:s0 + P], qT[0][:, SB:QW],
                            start=True, stop=True)
                        ext = expp.tile([P, QW], BF16, name="ext", tag="ex")
                        nc.scalar.activation(
                            ext[:, :], ps_s[:, :], ACT.Exp,
                            bias=mask_sb[:, st:st + 1], scale=1.0)
                        exs0[st] = ext
                        if st == 1:
                            nc.vector.tensor_add(
                                racc0[:, :], exs0[0][:, :], exs0[1][:, :])
                        elif st > 1:
                            nc.vector.tensor_add(
                                racc0[:, :], racc0[:, :], ext[:, :])

                # ---------------- Phase C: attention + out-projection -------
                with nc.named_scope("attn"), \
                     tc.tile_pool(name="expp", bufs=8) as expp, \
                     tc.tile_pool(name="raccp", bufs=2) as raccp, \
                     tc.tile_pool(name="rsump", bufs=2) as rsump, \
                     tc.tile_pool(name="rcpp", bufs=2) as rcpp, \
                     tc.tile_pool(name="otn", bufs=2) as otn, \
                     tc.tile_pool(name="ystg", bufs=2) as ystg, \
                     tc.tile_pool(name="ps2", bufs=2, space="PSUM") as ps2, \
                     tc.tile_pool(name="ps_pv", bufs=2, space="PSUM") as pspv:
                    def op_piece(qb, oTn, eo, on_dve):
                        # one out-projection eo-group: 8 matmuls into a ps2
                        # tile, evict, 4 DMA pieces across queues
                        q0 = qb * QW
                        yps = ps2.tile([P, QW], F32, name="yps", tag="ps2")
                        for half in range(2):
                            hs = half * SB
                            for dv in range(HPC):
                                nc.tensor.matmul(
                                    yps[:, hs:hs + SB],
                                    wo_sb[dv][:, eo * P:(eo + 1) * P],
                                    oTn[dv][:, hs:hs + SB],
                                    start=(dv == 0), stop=(dv == HPC - 1))
                        yst = ystg.tile([P, QW], BF16, name="yst", tag="yst")
                        nc.scalar.copy(yst[:, 0:SB], yps[:, 0:SB])
                        nc.vector.tensor_copy(yst[:, SB:QW], yps[:, SB:QW])
                        for piece in range(2):
                            pc = piece * (QW // 2)
                            nc.sync.dma_start(
                                yT[eo * P:(eo + 1) * P,
                                   q0 + pc:q0 + pc + QW // 2],
                                yst[:, pc:pc + QW // 2])

                    # out-projection eo-groups of block qb-1 are interleaved
                    # into block qb's attention: they fill the PE's deficit
                    # in the scalar-exp-paced sk loop (and keep the PE's
                    # p-state hot).  Head 0 gets one late piece (the last
                    # head of qb-1 must finish its r-chain first).
                    OP_SLOTS = {0: (14,), 1: (2, 5, 8, 11, 14),
                                2: (2, 5, 8, 11, 14), 3: (2, 5, 8, 11, 14)}

                    def attn_block(qb, h, oTn, op_work):
                        q0 = qb * QW
                        pv0 = pspv.tile([P, SB], F32, name="pv0", tag="pv0")
                        pv1 = pspv.tile([P, SB], F32, name="pv1", tag="pv1")
                        exs = [None] * ST
                        racc = raccp.tile([P, QW], BF16, name="racc",
                                          tag="racc")

                        def emit_pv(sk):
                            nc.tensor.matmul(
                                pv0[:, :],
                                vv[sk][:, h * P:(h + 1) * P],
                                exs[sk][:, 0:SB],
                                start=(sk == 0), stop=(sk == ST - 1))
                            nc.tensor.matmul(
                                pv1[:, :],
                                vv[sk][:, h * P:(h + 1) * P],
                                exs[sk][:, SB:QW],
                                start=(sk == 0), stop=(sk == ST - 1))

                        slots = OP_SLOTS[h] if op_work else ()
                        for sk in range(ST):
                            ps = ps2.tile([P, QW], F32, name="ps_sc",
                                          tag="ps2")
                            nc.tensor.matmul(
                                ps[:, 0:SB],
                                kT[h][:, sk * P:(sk + 1) * P],
                                qT[h][:, q0:q0 + SB],
                                start=True, stop=True)
                            nc.tensor.matmul(
                                ps[:, SB:QW],
                                kT[h][:, sk * P:(sk + 1) * P],
                                qT[h][:, q0 + SB:q0 + QW],
                                start=True, stop=True)
                            ext = expp.tile([P, QW], BF16, name="ext",
                                            tag="ex")
                            nc.scalar.activation(
                                ext[:, :], ps[:, :], ACT.Exp,
                                bias=mask_sb[:, sk:sk + 1], scale=1.0)
                            exs[sk] = ext
                            if sk == 1:
                                nc.vector.tensor_add(
                                    racc[:, :], exs[0][:, :], exs[1][:, :])
                            elif sk > 1:
                                nc.vector.tensor_add(
                                    racc[:, :], racc[:, :], ext[:, :])
                            if sk >= PIPE:
                                emit_pv(sk - PIPE)
                            if sk in slots and op_work:
                                pqb, poTn, peos = op_work
                                if peos:
                                    op_piece(pqb, poTn, peos.pop(0),
                                             on_dve=True)
                        for sk in range(ST - PIPE, ST):
                            emit_pv(sk)

                        # softmax denominator: one gpsimd sum+broadcast (a
                        # second call would pay a ~7us gpsimd drain), fast
                        # approximate reciprocal, normalize on eviction
                        rs = rsump.tile([P, QW], F32, name="rs", tag="rs")
                        nc.gpsimd.partition_all_reduce(
                            rs[:, :], racc[:, :], channels=P,
                            reduce_op=bass_isa.ReduceOp.add)
                        rc = rcpp.tile([P, QW], F32, name="rc", tag="rc")
                        nc.vector.reciprocal_approx_fast(rc[:, :], rs[:, :])
                        o = otn.tile([P, QW], BF16, name=f"oTn{h}",
                                     tag=f"oTn{h}")
                        nc.vector.tensor_mul(o[:, 0:SB], pv0[:, :],
                                             rc[:, 0:SB])
                        nc.vector.tensor_mul(o[:, SB:QW], pv1[:, :],
                                             rc[:, SB:QW])
                        oTn.append(o)

                    oTns = [[] for _ in range(NQB)]
                    for qb in range(NQB):
                        peos = list(range(DTI)) if qb > 0 else []
                        for h in range(HPC):
                            op_work = ((qb - 1, oTns[qb - 1], peos)
                                       if qb > 0 else None)
                            attn_block(qb, h, oTns[qb], op_work)
                    # final block's out-projection: straight burst, evictions
                    # alternating scalar/DVE
                    for eo in range(DTI):
                        op_piece(NQB - 1, oTns[NQB - 1], eo,
                                 on_dve=(eo % 2 == 1))

    nc.compile()
    return nc


_NC_CACHE = {}


def _get_nc():
    if "nc" not in _NC_CACHE:
        _NC_CACHE["nc"] = _build()
    return _NC_CACHE["nc"]


def kernel(hidden_states, attention_mask, Wq, bq, Wk, bk, Wv, bv, Wo, bo):
    hidden_states = np.asarray(hidden_states, dtype=np.float32)
    attention_mask = np.asarray(attention_mask, dtype=np.float32)
    Wq = np.asarray(Wq, dtype=np.float32)
    Wk = np.asarray(Wk, dtype=np.float32)
    Wv = np.asarray(Wv, dtype=np.float32)
    Wo = np.asarray(Wo, dtype=np.float32)
    bq = np.asarray(bq, dtype=np.float32)
    bk = np.asarray(bk, dtype=np.float32)
    bv = np.asarray(bv, dtype=np.float32)
    bo = np.asarray(bo, dtype=np.float32)

    nc = _get_nc()

    # Host-side sharding prep (cheap numpy work, not on the HW critical path)
    xTh = [np.ascontiguousarray(hidden_states[b].T).astype(BF16_NP)
           for b in range(B)]
    addmask = [np.ascontiguousarray((1.0 - attention_mask[b]) * MASK_MIN)
               for b in range(B)]
    ones = np.ones(P, dtype=BF16_NP)
    in_maps = []
    for c in range(N_CORES):
        b, g = c // 4, c % 4
        sl = slice(g * E, (g + 1) * E)
        im = {
            "xT": xTh[b],
            "wqT": np.ascontiguousarray((Wq[sl, :] * SCALE).T).astype(BF16_NP),
            "wkT": np.ascontiguousarray(Wk[sl, :].T).astype(BF16_NP),
            "wvT": np.ascontiguousarray(Wv[sl, :].T).astype(BF16_NP),
            "woT": np.ascontiguousarray(Wo[:, sl].T).astype(BF16_NP),
            "maskT": addmask[b],
            "bq": np.ascontiguousarray(bq[sl] * SCALE),
            "bk": np.ascontiguousarray(bk[sl]),
            "bv": np.ascontiguousarray(bv[sl]).astype(BF16_NP),
            "ones1": ones,
        }
        in_maps.append(im)

    res = bass_utils.run_bass_kernel_spmd(
        nc, in_maps, core_ids=list(range(N_CORES)),
        trace=bool(int(os.environ.get("BASS_KERNEL_TRACE", "0"))))
    kernel.last_results = res

    out = np.empty((B, S, D), dtype=np.float32)
    for b in range(B):
        acc = res.results[b * 4]["yT"].astype(np.float32)
        for g in range(1, 4):
            acc += res.results[b * 4 + g]["yT"].astype(np.float32)
        out[b] = acc.T + bo
    return out


# revision 28
# speedup vs baseline: 1.0603x; 1.0603x over previous
"""LlamaAttention (B=2, S=2048, D=2048, H=16) on 8 Trainium2 NeuronCores.

Sharding: batch x head-group. Core c handles batch b = c // 4 and head group
g = c % 4 (4 heads of 128 dims each -> a 512-wide slice of q/k/v space).
Each core computes q/k/v projections for its slice, attention for its 4
heads, and a partial out-projection (contracting only its 512 dv dims).
Host sums the 4 partials per batch and adds the output bias.

Design notes (engine rates measured from hardware traces):
  - PE is the roofline (1536 matmuls x 512 moving cols ~ 330 us/core); the
    schedule keeps the PE queue dense so it stays at full p-state (trn2 PE
    halves its clock after idle gaps and needs ~3us to ramp back).
  - Everything is bf16 on the device (same PE rate as float32r, half the
    DMA/SBUF, 2x DVE): x and the four weights are converted host-side
    (host time is not on the HW critical path).  Measured rel err ~5e-3
    against the fp32 reference, mostly from bf16 q/k logit noise.
  - x (8MB in bf16) is loaded ONCE and stays resident; q/k and v
    projections both read it from SBUF.  Total HBM traffic ~32MB.
  - x DMAs land chunk-major ([128,512] pieces of the resident [128,2048]
    tiles) so the first projection group starts after ~2MB, not 8MB.
  - q/k scale+bias folded into the PSUM eviction (activation bias; scale
    pre-folded into Wq host-side); v bias via a ones-row matmul into PSUM.
  - scores are computed transposed (keys on partitions) into 2-bank PSUM
    tiles, so exp is a single 1024-wide activation with the additive
    attention mask as its per-partition bias (exact for the general
    [B, S] mask since keys sit on partitions).
  - softmax denominator: bf16 racc accumulated on DVE (2x 16-bit mode),
    one gpsimd partition_all_reduce (sum+broadcast across partitions in
    one op), reciprocal on the SCALAR engine (DVE reciprocal is a 6.5us
    multi-pass op; scalar does it in ~1us), normalization fused into the
    PV-psum eviction on DVE.
  - attention is emitted per (query-block 1024, head); PV matmuls are
    software-pipelined 2 sk-steps behind the score matmuls; the
    out-projection shares the scores' 2-bank PSUM rotation so PSUM is
    exactly 8 banks.
  - the attention inner loop is scalar-exp-paced (~1.25us per 16K-score
    exp), so out-projection eo-groups of query-block qb-1 are interleaved
    into qb's sk loops to fill the PE's deficit, and head 0 of block 0
    gets its scores+exps precomputed during the (pure-PE) v projection.
  - y is evacuated as bf16 (host sums the 4 partials in fp32): full-rate
    fp32 y DMAs contend with the PE's SBUF operand fetches and slowed
    out-projection matmuls to ~427ns.
"""

import os
import numpy as np
import ml_dtypes

import concourse.bass as bass
import concourse.tile as tile
from concourse import bacc, mybir, bass_isa
from concourse import bass_utils

B, S, D = 2, 2048, 2048
NH, HD = 16, 128
N_CORES = 8
HPC = 4                      # heads per core
E = HPC * HD                 # 512: per-core q/k/v width
SCALE = float(HD) ** -0.5
F32 = mybir.dt.float32
BF16 = mybir.dt.bfloat16
MM_DT = BF16                 # matmul input dtype everywhere

P = 128                      # partition tile
ST = S // P                  # 16 s partition-tiles
DTI = D // P                 # 16 d partition-tiles
SB = 512                     # matmul moving-dim block
NCH = S // SB                # 4 s chunks for the projection passes
QW = 1024                    # attention query-block width (2-bank psum)
NQB = S // QW                # 2 query blocks
PIPE = 2                     # PV pipeline lag (sk steps)
MASK_MIN = float(np.finfo(np.float32).min)

BF16_NP = ml_dtypes.bfloat16


def _build():
    nc = bacc.Bacc("TRN2", target_bir_lowering=False, debug=False,
                   num_devices=N_CORES)

    xT = nc.dram_tensor("xT", [D, S], BF16, kind="ExternalInput").ap()
    wqT = nc.dram_tensor("wqT", [D, E], BF16, kind="ExternalInput").ap()
    wkT = nc.dram_tensor("wkT", [D, E], BF16, kind="ExternalInput").ap()
    wvT = nc.dram_tensor("wvT", [D, E], BF16, kind="ExternalInput").ap()
    woT = nc.dram_tensor("woT", [E, D], BF16, kind="ExternalInput").ap()
    maskT = nc.dram_tensor("maskT", [S], F32, kind="ExternalInput").ap()
    bqd = nc.dram_tensor("bq", [E], F32, kind="ExternalInput").ap()
    bkd = nc.dram_tensor("bk", [E], F32, kind="ExternalInput").ap()
    bvd = nc.dram_tensor("bv", [E], BF16, kind="ExternalInput").ap()
    ones1 = nc.dram_tensor("ones1", [P], BF16, kind="ExternalInput").ap()
    yT = nc.dram_tensor("yT", [D, S], BF16, kind="ExternalOutput").ap()

    ACT = mybir.ActivationFunctionType

    with tile.TileContext(nc) as tc:
        with tc.tile_pool(name="persist", bufs=1) as persist, \
             tc.tile_pool(name="xpool", bufs=1) as xpool:
            qT = [persist.tile([P, S], BF16, name=f"qT{h}", tag=f"qT{h}")
                  for h in range(HPC)]
            kT = [persist.tile([P, S], BF16, name=f"kT{h}", tag=f"kT{h}")
                  for h in range(HPC)]
            mask_sb = persist.tile([P, ST], F32, name="mask_sb", tag="mask")
            bq_sb = persist.tile([P, HPC], F32, name="bq_sb", tag="bq")
            bk_sb = persist.tile([P, HPC], F32, name="bk_sb", tag="bk")
            bv_row = persist.tile([1, E], BF16, name="bv_row", tag="bv")
            ones_rp = persist.tile([1, P], BF16, name="ones_rp", tag="onesr")
            nc.sync.dma_start(mask_sb[:, :],
                              maskT.rearrange("(t p) -> p t", p=P))
            nc.sync.dma_start(bq_sb[:, :],
                              bqd.rearrange("(t p) -> p t", p=P))
            nc.sync.dma_start(bk_sb[:, :],
                              bkd.rearrange("(t p) -> p t", p=P))
            nc.sync.dma_start(bv_row[:, :],
                              bvd.rearrange("(a e) -> a e", a=1))
            nc.sync.dma_start(ones_rp[:, :],
                              ones1.rearrange("(a e) -> a e", a=1))

            # Resident x: 16 tiles [128, 2048] bf16 (64KB/partition).
            xsb = [xpool.tile([P, S], BF16, name=f"x_{dt}", tag=f"x_{dt}")
                   for dt in range(DTI)]

            def load_x_chunk(ch):
                c0 = ch * SB
                for dt in range(DTI):
                    nc.sync.dma_start(xsb[dt][:, c0:c0 + SB],
                                      xT[dt * P:(dt + 1) * P, c0:c0 + SB])

            # ---------------- Phase A: q + k projections --------------------
            # qT[e, s] = wq.T-slice @ x (+bq, scale pre-folded); kT likewise.
            with nc.named_scope("proj_qk"), \
                 tc.tile_pool(name="wqk", bufs=1) as wpool, \
                 tc.tile_pool(name="ps_a", bufs=6, space="PSUM") as psa:
                wq_sb, wk_sb = [], []
                # interleave wq/x-chunk0 DMAs so chunk-0 matmuls start early
                for dt in range(DTI):
                    wq_t = wpool.tile([P, E], BF16, name=f"wq_{dt}",
                                      tag=f"wq_{dt}")
                    nc.sync.dma_start(wq_t[:, 0:P],
                                      wqT[dt * P:(dt + 1) * P, 0:P])
                    wq_sb.append(wq_t)
                    nc.sync.dma_start(xsb[dt][:, 0:SB],
                                      xT[dt * P:(dt + 1) * P, 0:SB])
                for dt in range(DTI):
                    nc.sync.dma_start(wq_sb[dt][:, P:E],
                                      wqT[dt * P:(dt + 1) * P, P:E])
                for dt in range(DTI):
                    wk_t = wpool.tile([P, E], BF16, name=f"wk_{dt}",
                                      tag=f"wk_{dt}")
                    nc.sync.dma_start(wk_t[:, :],
                                      wkT[dt * P:(dt + 1) * P, :])
                    wk_sb.append(wk_t)
                load_x_chunk(1)
                for ch in range(NCH):
                    c0 = ch * SB
                    if 2 <= ch + 2 < NCH:
                        load_x_chunk(ch + 2)
                    for wsb, outT, bsb in ((wq_sb, qT, bq_sb),
                                           (wk_sb, kT, bk_sb)):
                        for et in range(HPC):
                            ps = psa.tile([P, SB], F32, name="ps_at")
                            for dt in range(DTI):
                                nc.tensor.matmul(
                                    ps[:, :],
                                    wsb[dt][:, et * P:(et + 1) * P],
                                    xsb[dt][:, c0:c0 + SB],
                                    start=(dt == 0), stop=(dt == DTI - 1))
                            nc.scalar.activation(
                                outT[et][:, c0:c0 + SB], ps[:, :],
                                ACT.Identity, bias=bsb[:, et:et + 1])

            with tc.tile_pool(name="late", bufs=1) as late:
                vv = [late.tile([P, E], BF16, name=f"v{st}", tag=f"v{st}")
                      for st in range(ST)]
                wo_sb = [late.tile([P, D], BF16, name=f"wo_{dv}",
                                   tag=f"wo_{dv}") for dv in range(HPC)]

                # expp/raccp/ps2 open before phase B: the v projection is
                # pure PE work, so head 0 of query-block 0 gets its scores
                # and exps computed there (the scalar is otherwise idle),
                # collapsing its phase-C block to just the PV matmuls.
                exs0 = [None] * ST
                with tc.tile_pool(name="expp", bufs=18) as expp, \
                     tc.tile_pool(name="raccp", bufs=2) as raccp, \
                     tc.tile_pool(name="ps2", bufs=2, space="PSUM") as ps2:
                  racc0 = raccp.tile([P, QW], BF16, name="racc", tag="racc")
                  # ---------------- Phase B: v projection (natural) ---------
                  # v[s, e] = x-slice.T @ wv + bv; x already resident.
                  with nc.named_scope("proj_v"), \
                       tc.tile_pool(name="wv", bufs=1) as wvpool, \
                       tc.tile_pool(name="ps_v", bufs=4, space="PSUM") as psv:
                    wv_sb = []
                    for dt in range(DTI):
                        wv_t = wvpool.tile([P, E], BF16, name=f"wv_{dt}",
                                           tag=f"wv_{dt}")
                        nc.sync.dma_start(wv_t[:, :],
                                          wvT[dt * P:(dt + 1) * P, :])
                        wv_sb.append(wv_t)
                    for dv in range(HPC):
                        nc.sync.dma_start(wo_sb[dv][:, :],
                                          woT[dv * P:(dv + 1) * P, :])
                    for st in range(ST):
                        s0 = st * P
                        ps = psv.tile([P, E], F32, name="ps_vt")
                        for dt in range(DTI):
                            nc.tensor.matmul(
                                ps[:, :],
                                xsb[dt][:, s0:s0 + P],
                                wv_sb[dt][:, :],
                                start=(dt == 0), stop=False)
                        nc.tensor.matmul(
                            ps[:, :], ones_rp[0:1, :], bv_row[0:1, :],
                            start=False, stop=True)
                        nc.vector.tensor_copy(vv[st][:, :], ps[:, :])
                        # (qb0, h0) scores + exp ride along: scalar work
                        # under phase B's PE-bound v projection
                        ps_s = ps2.tile([P, QW], F32, name="ps_sc",
                                        tag="ps2")
                        nc.tensor.matmul(
                            ps_s[:, 0:SB],
                            kT[0][:, s0:s0 + P], qT[0][:, 0:SB],
                            start=True, stop=True)
                        nc.tensor.matmul(
                            ps_s[:, SB:QW],
                            kT[0][:, s0:s0 + P], qT[0][:, SB:QW],
                            start=True, stop=True)
                        ext = expp.tile([P, QW], BF16, name="ext", tag="ex")
                        nc.scalar.activation(
                            ext[:, :], ps_s[:, :], ACT.Exp,
                            bias=mask_sb[:, st:st + 1], scale=1.0)
                        exs0[st] = ext
                        if st == 1:
                            nc.vector.tensor_add(
                                racc0[:, :], exs0[0][:, :], exs0[1][:, :])
                        elif st > 1:
                            nc.vector.tensor_add(
                                racc0[:, :], racc0[:, :], ext[:, :])

                # ---------------- Phase C: attention + out-projection -------
                with nc.named_scope("attn"), \
                     tc.tile_pool(name="expp", bufs=8) as expp, \
                     tc.tile_pool(name="raccp", bufs=2) as raccp, \
                     tc.tile_pool(name="rsump", bufs=2) as rsump, \
                     tc.tile_pool(name="rcpp", bufs=2) as rcpp, \
                     tc.tile_pool(name="otn", bufs=2) as otn, \
                     tc.tile_pool(name="ystg", bufs=2) as ystg, \
                     tc.tile_pool(name="ps2", bufs=2, space="PSUM") as ps2, \
                     tc.tile_pool(name="ps_pv", bufs=2, space="PSUM") as pspv:
                    def op_piece(qb, oTn, eo, on_dve):
                        # one out-projection eo-group: 8 matmuls into a ps2
                        # tile, evict, 4 DMA pieces across queues
                        q0 = qb * QW
                        yps = ps2.tile([P, QW], F32, name="yps", tag="ps2")
                        for half in range(2):
                            hs = half * SB
                            for dv in range(HPC):
                                nc.tensor.matmul(
                                    yps[:, hs:hs + SB],
                                    wo_sb[dv][:, eo * P:(eo + 1) * P],
                                    oTn[dv][:, hs:hs + SB],
                                    start=(dv == 0), stop=(dv == HPC - 1))
                        yst = ystg.tile([P, QW], BF16, name="yst", tag="yst")
                        nc.scalar.copy(yst[:, 0:SB], yps[:, 0:SB])
                        nc.vector.tensor_copy(yst[:, SB:QW], yps[:, SB:QW])
                        for piece in range(2):
                            pc = piece * (QW // 2)
                            nc.sync.dma_start(
                                yT[eo * P:(eo + 1) * P,
                                   q0 + pc:q0 + pc + QW // 2],
                                yst[:, pc:pc + QW // 2])

                    # out-projection eo-groups of block qb-1 are interleaved
                    # into block qb's attention: they fill the PE's deficit
                    # in the scalar-exp-paced sk loop (and keep the PE's
                    # p-state hot).  Head 0 gets one late piece (the last
                    # head of qb-1 must finish its r-chain first).
                    OP_SLOTS = {0: (14,), 1: (2, 5, 8, 11, 14),
                                2: (2, 5, 8, 11, 14), 3: (2, 5, 8, 11, 14)}

                    def attn_block(qb, h, oTn, op_work):
                        q0 = qb * QW
                        pv0 = pspv.tile([P, SB], F32, name="pv0", tag="pv0")
                        pv1 = pspv.tile([P, SB], F32, name="pv1", tag="pv1")
                        exs = [None] * ST
                        racc = raccp.tile([P, QW], BF16, name="racc",
                                          tag="racc")

                        def emit_pv(sk):
                            nc.tensor.matmul(
                                pv0[:, :],
                                vv[sk][:, h * P:(h + 1) * P],
                                exs[sk][:, 0:SB],
                                start=(sk == 0), stop=(sk == ST - 1))
                            nc.tensor.matmul(
                                pv1[:, :],
                                vv[sk][:, h * P:(h + 1) * P],
                                exs[sk][:, SB:QW],
                                start=(sk == 0), stop=(sk == ST - 1))

                        slots = OP_SLOTS[h] if op_work else ()
                        for sk in range(ST):
                            ps = ps2.tile([P, QW], F32, name="ps_sc",
                                          tag="ps2")
                            nc.tensor.matmul(
                                ps[:, 0:SB],
                                kT[h][:, sk * P:(sk + 1) * P],
                                qT[h][:, q0:q0 + SB],
                                start=True, stop=True)
                            nc.tensor.matmul(
                                ps[:, SB:QW],
                                kT[h][:, sk * P:(sk + 1) * P],
                                qT[h][:, q0 + SB:q0 + QW],
                                start=True, stop=True)
                            ext = expp.tile([P, QW], BF16, name="ext",
                                            tag="ex")
                            nc.scalar.activation(
                                ext[:, :], ps[:, :], ACT.Exp,
                                bias=mask_sb[:, sk:sk + 1], scale=1.0)
                            exs[sk] = ext
                            if sk == 1:
                                nc.vector.tensor_add(
                                    racc[:, :], exs[0][:, :], exs[1][:, :])
                            elif sk > 1:
                                nc.vector.tensor_add(
                                    racc[:, :], racc[:, :], ext[:, :])
                            if sk >= PIPE:
                                emit_pv(sk - PIPE)
                            if sk in slots and op_work:
                                pqb, poTn, peos = op_work
                                if peos:
                                    op_piece(pqb, poTn, peos.pop(0),
                                             on_dve=True)
                        for sk in range(ST - PIPE, ST):
                            emit_pv(sk)

                        # softmax denominator: one gpsimd sum+broadcast (a
                        # second call would pay a ~7us gpsimd drain), fast
                        # approximate reciprocal, normalize on eviction
                        rs = rsump.tile([P, QW], F32, name="rs", tag="rs")
                        nc.gpsimd.partition_all_reduce(
                            rs[:, :], racc[:, :], channels=P,
                            reduce_op=bass_isa.ReduceOp.add)
                        rc = rcpp.tile([P, QW], F32, name="rc", tag="rc")
                        nc.vector.reciprocal_approx_fast(rc[:, :], rs[:, :])
                        o = otn.tile([P, QW], BF16, name=f"oTn{h}",
                                     tag=f"oTn{h}")
                        nc.vector.tensor_mul(o[:, 0:SB], pv0[:, :],
                                             rc[:, 0:SB])
                        nc.vector.tensor_mul(o[:, SB:QW], pv1[:, :],
                                             rc[:, SB:QW])
                        oTn.append(o)

                    oTns = [[] for _ in range(NQB)]
                    for qb in range(NQB):
                        peos = list(range(DTI)) if qb > 0 else []
                        for h in range(HPC):
                            op_work = ((qb - 1, oTns[qb - 1], peos)
                                       if qb > 0 else None)
                            attn_block(qb, h, oTns[qb], op_work)
                    # final block's out-projection: straight burst, evictions
                    # alternating scalar/DVE
                    for eo in range(DTI):
                        op_piece(NQB - 1, oTns[NQB - 1], eo,
                                 on_dve=(eo % 2 == 1))

    nc.compile()
    return nc


_NC_CACHE = {}


def _get_nc():
    if "nc" not in _NC_CACHE:
        _NC_CACHE["nc"] = _build()
    return _NC_CACHE["nc"]


def kernel(hidden_states, attention_mask, Wq, bq, Wk, bk, Wv, bv, Wo, bo):
    hidden_states = np.asarray(hidden_states, dtype=np.float32)
    attention_mask = np.asarray(attention_mask, dtype=np.float32)
    Wq = np.asarray(Wq, dtype=np.float32)
    Wk = np.asarray(Wk, dtype=np.float32)
    Wv = np.asarray(Wv, dtype=np.float32)
    Wo = np.asarray(Wo, dtype=np.float32)
    bq = np.asarray(bq, dtype=np.float32)
    bk = np.asarray(bk, dtype=np.float32)
    bv = np.asarray(bv, dtype=np.float32)
    bo = np.asarray(bo, dtype=np.float32)

    nc = _get_nc()

    # Host-side sharding prep (cheap numpy work, not on the HW critical path)
    xTh = [np.ascontiguousarray(hidden_states[b].T).astype(BF16_NP)
           for b in range(B)]
    addmask = [np.ascontiguousarray((1.0 - attention_mask[b]) * MASK_MIN)
               for b in range(B)]
    ones = np.ones(P, dtype=BF16_NP)
    in_maps = []
    for c in range(N_CORES):
        b, g = c // 4, c % 4
        sl = slice(g * E, (g + 1) * E)
        im = {
            "xT": xTh[b],
            "wqT": np.ascontiguousarray((Wq[sl, :] * SCALE).T).astype(BF16_NP),
            "wkT": np.ascontiguousarray(Wk[sl, :].T).astype(BF16_NP),
            "wvT": np.ascontiguousarray(Wv[sl, :].T).astype(BF16_NP),
            "woT": np.ascontiguousarray(Wo[:, sl].T).astype(BF16_NP),
            "maskT": addmask[b],
            "bq": np.ascontiguousarray(bq[sl] * SCALE),
            "bk": np.ascontiguousarray(bk[sl]),
            "bv": np.ascontiguousarray(bv[sl]).astype(BF16_NP),
            "ones1": ones,
        }
        in_maps.append(im)

    res = bass_utils.run_bass_kernel_spmd(
        nc, in_maps, core_ids=list(range(N_CORES)),
        trace=bool(int(os.environ.get("BASS_KERNEL_TRACE", "0"))))
    kernel.last_results = res

    out = np.empty((B, S, D), dtype=np.float32)
    for b in range(B):
        acc = res.results[b * 4]["yT"].astype(np.float32)
        for g in range(1, 4):
            acc += res.results[b * 4 + g]["yT"].astype(np.float32)
        out[b] = acc.T + bo
    return out


# revision 29
# speedup vs baseline: 1.0672x; 1.0066x over previous
"""LlamaAttention (B=2, S=2048, D=2048, H=16) on 8 Trainium2 NeuronCores.

Sharding: batch x head-group. Core c handles batch b = c // 4 and head group
g = c % 4 (4 heads of 128 dims each -> a 512-wide slice of q/k/v space).
Each core computes q/k/v projections for its slice, attention for its 4
heads, and a partial out-projection (contracting only its 512 dv dims).
Host sums the 4 partials per batch and adds the output bias.

Design notes (engine rates measured from hardware traces):
  - PE is the roofline (1536 matmuls x 512 moving cols ~ 330 us/core); the
    schedule keeps the PE queue dense so it stays at full p-state (trn2 PE
    halves its clock after idle gaps and needs ~3us to ramp back).
  - Everything is bf16 on the device (same PE rate as float32r, half the
    DMA/SBUF, 2x DVE): x and the four weights are converted host-side
    (host time is not on the HW critical path).  Measured rel err ~5e-3
    against the fp32 reference, mostly from bf16 q/k logit noise.
  - x (8MB in bf16) is loaded ONCE and stays resident; q/k and v
    projections both read it from SBUF.  Total HBM traffic ~32MB.
  - x DMAs land chunk-major ([128,512] pieces of the resident [128,2048]
    tiles) so the first projection group starts after ~2MB, not 8MB.
  - q/k scale+bias folded into the PSUM eviction (activation bias; scale
    pre-folded into Wq host-side); v bias via a ones-row matmul into PSUM.
  - scores are computed transposed (keys on partitions) into 2-bank PSUM
    tiles, so exp is a single 1024-wide activation with the additive
    attention mask as its per-partition bias (exact for the general
    [B, S] mask since keys sit on partitions).
  - softmax denominator: bf16 racc accumulated on DVE (2x 16-bit mode),
    one gpsimd partition_all_reduce (sum+broadcast across partitions in
    one op), reciprocal on the SCALAR engine (DVE reciprocal is a 6.5us
    multi-pass op; scalar does it in ~1us), normalization fused into the
    PV-psum eviction on DVE.
  - attention is emitted per (query-block 1024, head); PV matmuls are
    software-pipelined 2 sk-steps behind the score matmuls; the
    out-projection shares the scores' 2-bank PSUM rotation so PSUM is
    exactly 8 banks.
  - the attention inner loop is scalar-exp-paced (~1.25us per 16K-score
    exp), so out-projection eo-groups of query-block qb-1 are interleaved
    into qb's sk loops to fill the PE's deficit, and head 0 of block 0
    gets its scores+exps precomputed during the (pure-PE) v projection.
  - y is evacuated as bf16 (host sums the 4 partials in fp32): full-rate
    fp32 y DMAs contend with the PE's SBUF operand fetches and slowed
    out-projection matmuls to ~427ns.
"""

import os
import numpy as np
import ml_dtypes

import concourse.bass as bass
import concourse.tile as tile
from concourse import bacc, mybir, bass_isa
from concourse import bass_utils

B, S, D = 2, 2048, 2048
NH, HD = 16, 128
N_CORES = 8
HPC = 4                      # heads per core
E = HPC * HD                 # 512: per-core q/k/v width
SCALE = float(HD) ** -0.5
F32 = mybir.dt.float32
BF16 = mybir.dt.bfloat16
MM_DT = BF16                 # matmul input dtype everywhere

P = 128                      # partition tile
ST = S // P                  # 16 s partition-tiles
DTI = D // P                 # 16 d partition-tiles
SB = 512                     # matmul moving-dim block
NCH = S // SB                # 4 s chunks for the projection passes
QW = 1024                    # attention query-block width (2-bank psum)
NQB = S // QW                # 2 query blocks
PIPE = 2                     # PV pipeline lag (sk steps)
MASK_MIN = float(np.finfo(np.float32).min)

BF16_NP = ml_dtypes.bfloat16


def _build(use_mask):
    nc = bacc.Bacc("TRN2", target_bir_lowering=False, debug=False,
                   num_devices=N_CORES)

    xT = nc.dram_tensor("xT", [D, S], BF16, kind="ExternalInput").ap()
    wqT = nc.dram_tensor("wqT", [D, E], BF16, kind="ExternalInput").ap()
    wkT = nc.dram_tensor("wkT", [D, E], BF16, kind="ExternalInput").ap()
    wvT = nc.dram_tensor("wvT", [D, E], BF16, kind="ExternalInput").ap()
    woT = nc.dram_tensor("woT", [E, D], BF16, kind="ExternalInput").ap()
    maskT = nc.dram_tensor("maskT", [S], F32, kind="ExternalInput").ap()
    bqd = nc.dram_tensor("bq", [E], F32, kind="ExternalInput").ap()
    bkd = nc.dram_tensor("bk", [E], F32, kind="ExternalInput").ap()
    bvd = nc.dram_tensor("bv", [E], BF16, kind="ExternalInput").ap()
    ones1 = nc.dram_tensor("ones1", [P], BF16, kind="ExternalInput").ap()
    yT = nc.dram_tensor("yT", [D, S], BF16, kind="ExternalOutput").ap()

    ACT = mybir.ActivationFunctionType

    with tile.TileContext(nc) as tc:
        with tc.tile_pool(name="persist", bufs=1) as persist, \
             tc.tile_pool(name="xpool", bufs=1) as xpool:
            qT = [persist.tile([P, S], BF16, name=f"qT{h}", tag=f"qT{h}")
                  for h in range(HPC)]
            kT = [persist.tile([P, S], BF16, name=f"kT{h}", tag=f"kT{h}")
                  for h in range(HPC)]
            mask_sb = persist.tile([P, ST], F32, name="mask_sb", tag="mask")
            bq_sb = persist.tile([P, HPC], F32, name="bq_sb", tag="bq")
            bk_sb = persist.tile([P, HPC], F32, name="bk_sb", tag="bk")
            bv_row = persist.tile([1, E], BF16, name="bv_row", tag="bv")
            ones_rp = persist.tile([1, P], BF16, name="ones_rp", tag="onesr")
            nc.sync.dma_start(mask_sb[:, :],
                              maskT.rearrange("(t p) -> p t", p=P))
            nc.sync.dma_start(bq_sb[:, :],
                              bqd.rearrange("(t p) -> p t", p=P))
            nc.sync.dma_start(bk_sb[:, :],
                              bkd.rearrange("(t p) -> p t", p=P))
            nc.sync.dma_start(bv_row[:, :],
                              bvd.rearrange("(a e) -> a e", a=1))
            nc.sync.dma_start(ones_rp[:, :],
                              ones1.rearrange("(a e) -> a e", a=1))

            # Resident x: 16 tiles [128, 2048] bf16 (64KB/partition).
            xsb = [xpool.tile([P, S], BF16, name=f"x_{dt}", tag=f"x_{dt}")
                   for dt in range(DTI)]

            def load_x_chunk(ch):
                c0 = ch * SB
                for dt in range(DTI):
                    nc.sync.dma_start(xsb[dt][:, c0:c0 + SB],
                                      xT[dt * P:(dt + 1) * P, c0:c0 + SB])

            # ---------------- Phase A: q + k projections --------------------
            # qT[e, s] = wq.T-slice @ x (+bq, scale pre-folded); kT likewise.
            with nc.named_scope("proj_qk"), \
                 tc.tile_pool(name="wqk", bufs=1) as wpool, \
                 tc.tile_pool(name="ps_a", bufs=6, space="PSUM") as psa:
                wq_sb, wk_sb = [], []
                # interleave wq/x-chunk0 DMAs so chunk-0 matmuls start early
                for dt in range(DTI):
                    wq_t = wpool.tile([P, E], BF16, name=f"wq_{dt}",
                                      tag=f"wq_{dt}")
                    nc.sync.dma_start(wq_t[:, 0:P],
                                      wqT[dt * P:(dt + 1) * P, 0:P])
                    wq_sb.append(wq_t)
                    nc.sync.dma_start(xsb[dt][:, 0:SB],
                                      xT[dt * P:(dt + 1) * P, 0:SB])
                for dt in range(DTI):
                    nc.sync.dma_start(wq_sb[dt][:, P:E],
                                      wqT[dt * P:(dt + 1) * P, P:E])
                for dt in range(DTI):
                    wk_t = wpool.tile([P, E], BF16, name=f"wk_{dt}",
                                      tag=f"wk_{dt}")
                    nc.sync.dma_start(wk_t[:, :],
                                      wkT[dt * P:(dt + 1) * P, :])
                    wk_sb.append(wk_t)
                load_x_chunk(1)
                for ch in range(NCH):
                    c0 = ch * SB
                    if 2 <= ch + 2 < NCH:
                        load_x_chunk(ch + 2)
                    for wsb, outT, bsb in ((wq_sb, qT, bq_sb),
                                           (wk_sb, kT, bk_sb)):
                        for et in range(HPC):
                            ps = psa.tile([P, SB], F32, name="ps_at")
                            for dt in range(DTI):
                                nc.tensor.matmul(
                                    ps[:, :],
                                    wsb[dt][:, et * P:(et + 1) * P],
                                    xsb[dt][:, c0:c0 + SB],
                                    start=(dt == 0), stop=(dt == DTI - 1))
                            nc.scalar.activation(
                                outT[et][:, c0:c0 + SB], ps[:, :],
                                ACT.Identity, bias=bsb[:, et:et + 1])

            with tc.tile_pool(name="late", bufs=1) as late:
                vv = [late.tile([P, E], BF16, name=f"v{st}", tag=f"v{st}")
                      for st in range(ST)]
                wo_sb = [late.tile([P, D], BF16, name=f"wo_{dv}",
                                   tag=f"wo_{dv}") for dv in range(HPC)]

                # expp/raccp/ps2 open before phase B: the v projection is
                # pure PE work, so head 0 of query-block 0 gets its scores
                # and exps computed there (the scalar is otherwise idle),
                # collapsing its phase-C block to just the PV matmuls.
                exs0 = [None] * ST
                with tc.tile_pool(name="expp", bufs=18) as expp, \
                     tc.tile_pool(name="raccp", bufs=2) as raccp, \
                     tc.tile_pool(name="ps2", bufs=2, space="PSUM") as ps2:
                  racc0 = raccp.tile([P, QW], BF16, name="racc", tag="racc")
                  # ---------------- Phase B: v projection (natural) ---------
                  # v[s, e] = x-slice.T @ wv + bv; x already resident.
                  with nc.named_scope("proj_v"), \
                       tc.tile_pool(name="wv", bufs=1) as wvpool, \
                       tc.tile_pool(name="ps_v", bufs=4, space="PSUM") as psv:
                    wv_sb = []
                    for dt in range(DTI):
                        wv_t = wvpool.tile([P, E], BF16, name=f"wv_{dt}",
                                           tag=f"wv_{dt}")
                        nc.sync.dma_start(wv_t[:, :],
                                          wvT[dt * P:(dt + 1) * P, :])
                        wv_sb.append(wv_t)
                    for dv in range(HPC):
                        nc.sync.dma_start(wo_sb[dv][:, :],
                                          woT[dv * P:(dv + 1) * P, :])
                    for st in range(ST):
                        s0 = st * P
                        ps = psv.tile([P, E], F32, name="ps_vt")
                        for dt in range(DTI):
                            nc.tensor.matmul(
                                ps[:, :],
                                xsb[dt][:, s0:s0 + P],
                                wv_sb[dt][:, :],
                                start=(dt == 0), stop=False)
                        nc.tensor.matmul(
                            ps[:, :], ones_rp[0:1, :], bv_row[0:1, :],
                            start=False, stop=True)
                        nc.vector.tensor_copy(vv[st][:, :], ps[:, :])
                        # (qb0, h0) scores + exp ride along: scalar work
                        # under phase B's PE-bound v projection
                        ps_s = ps2.tile([P, QW], F32, name="ps_sc",
                                        tag="ps2")
                        nc.tensor.matmul(
                            ps_s[:, 0:SB],
                            kT[0][:, s0:s0 + P], qT[0][:, 0:SB],
                            start=True, stop=True)
                        nc.tensor.matmul(
                            ps_s[:, SB:QW],
                            kT[0][:, s0:s0 + P], qT[0][:, SB:QW],
                            start=True, stop=True)
                        ext = expp.tile([P, QW], BF16, name="ext", tag="ex")
                        nc.scalar.activation(
                            ext[:, :], ps_s[:, :], ACT.Exp,
                            bias=(mask_sb[:, st:st + 1]
                                  if use_mask else 0.0), scale=1.0)
                        exs0[st] = ext
                        if st == 1:
                            nc.vector.tensor_add(
                                racc0[:, :], exs0[0][:, :], exs0[1][:, :])
                        elif st > 1:
                            nc.vector.tensor_add(
                                racc0[:, :], racc0[:, :], ext[:, :])

                # ---------------- Phase C: attention + out-projection -------
                with nc.named_scope("attn"), \
                     tc.tile_pool(name="expp", bufs=8) as expp, \
                     tc.tile_pool(name="raccp", bufs=2) as raccp, \
                     tc.tile_pool(name="rsump", bufs=2) as rsump, \
                     tc.tile_pool(name="rcpp", bufs=2) as rcpp, \
                     tc.tile_pool(name="otn", bufs=2) as otn, \
                     tc.tile_pool(name="ystg", bufs=2) as ystg, \
                     tc.tile_pool(name="ps2", bufs=2, space="PSUM") as ps2, \
                     tc.tile_pool(name="ps_pv", bufs=2, space="PSUM") as pspv:
                    def op_piece(qb, oTn, eo, on_dve):
                        # one out-projection eo-group: 8 matmuls into a ps2
                        # tile, evict, 4 DMA pieces across queues
                        q0 = qb * QW
                        yps = ps2.tile([P, QW], F32, name="yps", tag="ps2")
                        for half in range(2):
                            hs = half * SB
                            for dv in range(HPC):
                                nc.tensor.matmul(
                                    yps[:, hs:hs + SB],
                                    wo_sb[dv][:, eo * P:(eo + 1) * P],
                                    oTn[dv][:, hs:hs + SB],
                                    start=(dv == 0), stop=(dv == HPC - 1))
                        yst = ystg.tile([P, QW], BF16, name="yst", tag="yst")
                        nc.scalar.copy(yst[:, 0:SB], yps[:, 0:SB])
                        nc.vector.tensor_copy(yst[:, SB:QW], yps[:, SB:QW])
                        for piece in range(2):
                            pc = piece * (QW // 2)
                            nc.sync.dma_start(
                                yT[eo * P:(eo + 1) * P,
                                   q0 + pc:q0 + pc + QW // 2],
                                yst[:, pc:pc + QW // 2])

                    # out-projection eo-groups of block qb-1 are interleaved
                    # into block qb's attention: they fill the PE's deficit
                    # in the scalar-exp-paced sk loop (and keep the PE's
                    # p-state hot).  Head 0 gets one late piece (the last
                    # head of qb-1 must finish its r-chain first).
                    OP_SLOTS = {0: (14,), 1: (2, 5, 8, 11, 14),
                                2: (2, 5, 8, 11, 14), 3: (2, 5, 8, 11, 14)}

                    def attn_block(qb, h, oTn, op_work):
                        q0 = qb * QW
                        pv0 = pspv.tile([P, SB], F32, name="pv0", tag="pv0")
                        pv1 = pspv.tile([P, SB], F32, name="pv1", tag="pv1")
                        exs = [None] * ST
                        racc = raccp.tile([P, QW], BF16, name="racc",
                                          tag="racc")

                        def emit_pv(sk):
                            nc.tensor.matmul(
                                pv0[:, :],
                                vv[sk][:, h * P:(h + 1) * P],
                                exs[sk][:, 0:SB],
                                start=(sk == 0), stop=(sk == ST - 1))
                            nc.tensor.matmul(
                                pv1[:, :],
                                vv[sk][:, h * P:(h + 1) * P],
                                exs[sk][:, SB:QW],
                                start=(sk == 0), stop=(sk == ST - 1))

                        slots = OP_SLOTS[h] if op_work else ()
                        for sk in range(ST):
                            ps = ps2.tile([P, QW], F32, name="ps_sc",
                                          tag="ps2")
                            nc.tensor.matmul(
                                ps[:, 0:SB],
                                kT[h][:, sk * P:(sk + 1) * P],
                                qT[h][:, q0:q0 + SB],
                                start=True, stop=True)
                            nc.tensor.matmul(
                                ps[:, SB:QW],
                                kT[h][:, sk * P:(sk + 1) * P],
                                qT[h][:, q0 + SB:q0 + QW],
                                start=True, stop=True)
                            ext = expp.tile([P, QW], BF16, name="ext",
                                            tag="ex")
                            nc.scalar.activation(
                                ext[:, :], ps[:, :], ACT.Exp,
                                bias=mask_sb[:, sk:sk + 1], scale=1.0)
                            exs[sk] = ext
                            if sk == 1:
                                nc.vector.tensor_add(
                                    racc[:, :], exs[0][:, :], exs[1][:, :])
                            elif sk > 1:
                                nc.vector.tensor_add(
                                    racc[:, :], racc[:, :], ext[:, :])
                            if sk >= PIPE:
                                emit_pv(sk - PIPE)
                            if sk in slots and op_work:
                                pqb, poTn, peos = op_work
                                if peos:
                                    op_piece(pqb, poTn, peos.pop(0),
                                             on_dve=True)
                        for sk in range(ST - PIPE, ST):
                            emit_pv(sk)

                        # softmax denominator: one gpsimd sum+broadcast (a
                        # second call would pay a ~7us gpsimd drain), fast
                        # approximate reciprocal, normalize on eviction
                        rs = rsump.tile([P, QW], F32, name="rs", tag="rs")
                        nc.gpsimd.partition_all_reduce(
                            rs[:, :], racc[:, :], channels=P,
                            reduce_op=bass_isa.ReduceOp.add)
                        rc = rcpp.tile([P, QW], F32, name="rc", tag="rc")
                        nc.vector.reciprocal_approx_fast(rc[:, :], rs[:, :])
                        o = otn.tile([P, QW], BF16, name=f"oTn{h}",
                                     tag=f"oTn{h}")
                        nc.vector.tensor_mul(o[:, 0:SB], pv0[:, :],
                                             rc[:, 0:SB])
                        nc.vector.tensor_mul(o[:, SB:QW], pv1[:, :],
                                             rc[:, SB:QW])
                        oTn.append(o)

                    oTns = [[] for _ in range(NQB)]
                    for qb in range(NQB):
                        peos = list(range(DTI)) if qb > 0 else []
                        for h in range(HPC):
                            op_work = ((qb - 1, oTns[qb - 1], peos)
                                       if qb > 0 else None)
                            attn_block(qb, h, oTns[qb], op_work)
                    # final block's out-projection: straight burst, evictions
                    # alternating scalar/DVE
                    for eo in range(DTI):
                        op_piece(NQB - 1, oTns[NQB - 1], eo,
                                 on_dve=(eo % 2 == 1))

    nc.compile()
    return nc


_NC_CACHE = {}


def _get_nc(use_mask):
    if use_mask not in _NC_CACHE:
        _NC_CACHE[use_mask] = _build(use_mask)
    return _NC_CACHE[use_mask]


def kernel(hidden_states, attention_mask, Wq, bq, Wk, bk, Wv, bv, Wo, bo):
    hidden_states = np.asarray(hidden_states, dtype=np.float32)
    attention_mask = np.asarray(attention_mask, dtype=np.float32)
    Wq = np.asarray(Wq, dtype=np.float32)
    Wk = np.asarray(Wk, dtype=np.float32)
    Wv = np.asarray(Wv, dtype=np.float32)
    Wo = np.asarray(Wo, dtype=np.float32)
    bq = np.asarray(bq, dtype=np.float32)
    bk = np.asarray(bk, dtype=np.float32)
    bv = np.asarray(bv, dtype=np.float32)
    bo = np.asarray(bo, dtype=np.float32)

    use_mask = bool(np.any(attention_mask != 1.0))
    nc = _get_nc(use_mask)

    # Host-side sharding prep (cheap numpy work, not on the HW critical path)
    xTh = [np.ascontiguousarray(hidden_states[b].T).astype(BF16_NP)
           for b in range(B)]
    addmask = [np.ascontiguousarray((1.0 - attention_mask[b]) * MASK_MIN)
               for b in range(B)]
    ones = np.ones(P, dtype=BF16_NP)
    in_maps = []
    for c in range(N_CORES):
        b, g = c // 4, c % 4
        sl = slice(g * E, (g + 1) * E)
        im = {
            "xT": xTh[b],
            "wqT": np.ascontiguousarray((Wq[sl, :] * SCALE).T).astype(BF16_NP),
            "wkT": np.ascontiguousarray(Wk[sl, :].T).astype(BF16_NP),
            "wvT": np.ascontiguousarray(Wv[sl, :].T).astype(BF16_NP),
            "woT": np.ascontiguousarray(Wo[:, sl].T).astype(BF16_NP),
            "maskT": addmask[b],
            "bq": np.ascontiguousarray(bq[sl] * SCALE),
            "bk": np.ascontiguousarray(bk[sl]),
            "bv": np.ascontiguousarray(bv[sl]).astype(BF16_NP),
            "ones1": ones,
        }
        in_maps.append(im)

    res = bass_utils.run_bass_kernel_spmd(
        nc, in_maps, core_ids=list(range(N_CORES)),
        trace=bool(int(os.environ.get("BASS_KERNEL_TRACE", "0"))))
    kernel.last_results = res

    out = np.empty((B, S, D), dtype=np.float32)
    for b in range(B):
        acc = res.results[b * 4]["yT"].astype(np.float32)
        for g in range(1, 4):
            acc += res.results[b * 4 + g]["yT"].astype(np.float32)
        out[b] = acc.T + bo
    return out
